# revision 15
# baseline (speedup 1.0000x reference)
"""DMSAD loss kernel for Trainium2 (8 NeuronCores, data-parallel over batch).

Computes mean over B rows of:
    dist_i = max(min_j ||x_i - c_j||^2, 0)
    loss_i = dist_i                 if st_i == 0
             dist_i + EPS           if st_i == 1
             1 / (dist_i + EPS)     if st_i == -1

Strategy per core (B_SH = 16384 rows, D = 256, C = 128):
  - HBM load of x fp32 (16.8 MB/core, ~47us at 358 GB/s -- the roofline).
  - cast fp32->bf16 (engine knob: gpsimd/dve/act).
  - PE-transpose 128x128 bf16 chunks via matmul against identity (stays
    HAM-warm, FWL weight loads); PSUM->SBUF cast-copy on ACT/DVE (knob).
  - bf16 matmuls vs cT accumulate G = -2 x.c^T + c2 in PSUM fp32 (ones x
    [c2_hi; c2_lo] K=2 matmul folds the center norms in).
  - x2 row sums: batched squares pass (ACT Square or DVE STT, knob) +
    batched DVE tensor_reduce add.
  - DVE min-reduce over centers per PSUM group; endgame on [128, NT]
    tiles; ones-matmul partition reduction to one scalar.
Host sums the 8 per-core partial sums and divides by global B.
"""

from contextlib import ExitStack, nullcontext

import numpy as np

import concourse.bass as bass
import concourse.tile as tile
from concourse import bacc, mybir
from concourse.bass_utils import run_bass_kernel_spmd
from concourse.masks import make_identity

N_CORES = 8
B = 131072
D = 256
C = 128
P = 128
B_SH = B // N_CORES          # 16384 rows per core
NT = B_SH // P               # 128 b-tiles of 128 rows
PSUM_GROUP = 4               # b-tiles per PSUM batch (one G bank)
DMA_GROUP = 8                # b-tiles per input DMA (1 MiB fp32 reads)
ETA = 1.0
EPS = 1e-6

# ---- engine-balance knobs -------------------------------------------------
# fp32->bf16 cast of x, per 16 DMA groups: measured costs per [128,2048]
# group: DVE 1.13us (2x_2P), ACT 2.0us, GPSIMD 7.0us (bad Q7 ucode --
# only worth using for a couple of groups as a third lane).
CAST_ACT_OF_16 = 10
CAST_GPS_OF_16 = 0
# squares pass for x2 (out=bf16 sq tensor): "act" Square (from fp32 xf,
# no cast dependency) or "dve" scalar_tensor_tensor (bf16 2x from x8).
SQ_ENG = "act"
# x2 mode: "sqred" = batched sq pass + batched DVE reduce-add;
# "tile_accum" = legacy per-tile accum_out ops (X2_DVE_OF_10 splits them).
X2_MODE = "sqred"
X2_DVE_OF_10 = 3
# Of every 16 DMA groups, this many compute x2 ON THE PE: square the
# transposed tiles (DVE STT, bf16 2x) and accumulate a rank-1 ones-matmul
# into G so dist = min_c(G) directly -- trades ~1.4us/group of DVE reduce
# for ~1.3us/group of PE matmul + 0.7us/group of DVE STT, and keeps the
# PE dense enough to hold the HAM clock at 2.4 GHz.
X2_PE_OF_16 = 11
# PSUM->SBUF copy of transposed x: of every 8 PSUM groups, this many go to
# DVE (tensor_copy, ~1.21us/group), rest to ACT (scalar.copy, ~1.15us/group).
COPY_DVE_OF_8 = 0
# transpose x on the DMA xbar instead of the PE (the old baseline path;
# measured: doubles DMA engine time -> DMA-bound at ~107us).
TPOSE_DMA = False

F32 = mybir.dt.float32
BF16 = mybir.dt.bfloat16
AF = mybir.ActivationFunctionType
ALU = mybir.AluOpType

_cached_nc = {}


def _emit(ctx: ExitStack, tc, x_d, c_d, st_d, out_d, repeat: int = 1,
          hw_loop: int = 1):
    nc = tc.nc

    const = ctx.enter_context(tc.tile_pool(name="const", bufs=1))
    xpool = ctx.enter_context(tc.tile_pool(name="xin", bufs=5))
    xbpool = ctx.enter_context(tc.tile_pool(name="xb", bufs=3))
    sqpool = ctx.enter_context(tc.tile_pool(name="sq", bufs=2))
    xtps = ctx.enter_context(tc.tile_pool(name="xtps", bufs=2, space="PSUM"))
    xtsb = ctx.enter_context(tc.tile_pool(name="xtsb", bufs=3))
    # G lives in 2-bank [P, 8, C] tiles so the min-reduce batches a whole
    # DMA group.
    gps = ctx.enter_context(tc.tile_pool(name="gps", bufs=2, space="PSUM"))
    scr_ps = ctx.enter_context(tc.tile_pool(name="scrps", bufs=1, space="PSUM"))
    endp = ctx.enter_context(tc.tile_pool(name="endp", bufs=1))

    # ---- one-time prep -------------------------------------------------
    ident_bf = const.tile([P, P], BF16)
    make_identity(nc, ident_bf[:])
    ident_f32 = const.tile([P, P], F32)
    make_identity(nc, ident_f32[:])

    c_sb = const.tile([C, D], F32)
    nc.scalar.dma_start(c_sb[:], c_d[:])

    # c2 = rowsum(c^2) as a [128, 1] fp32 column
    c_sq = const.tile([C, D], F32)
    c2col = const.tile([C, 1], F32)
    nc.scalar.activation(c_sq[:], c_sb[:], AF.Square, accum_out=c2col[:])

    # (-2c) in bf16, then its transpose cT [d-chunk partitions, k, centers]
    cm2 = const.tile([C, D], BF16)
    nc.vector.tensor_scalar_mul(cm2[:], c_sb[:], -2.0)
    ct_ps = scr_ps.tile([P, 2, C], BF16, tag="scratch")
    for k in range(2):
        nc.tensor.transpose(ct_ps[:, k, :], cm2[:, k * P:(k + 1) * P], ident_bf[:])
    cT = const.tile([P, 2, C], BF16)
    nc.vector.tensor_copy(cT[:], ct_ps[:])

    # c2 as two bf16 K-rows (hi + lo) so a K=2 ones-matmul adds fp32-accurate c2
    c2t_ps = scr_ps.tile([1, C], F32, tag="scratch")
    nc.tensor.transpose(c2t_ps[:], c2col[:], ident_f32[:])
    c2row_f = const.tile([1, C], F32)
    nc.vector.tensor_copy(c2row_f[:], c2t_ps[:])
    c2rows = const.tile([2, C], BF16)
    nc.vector.tensor_copy(c2rows[0:1, :], c2row_f[:])
    c2hi_f = const.tile([1, C], F32)
    nc.vector.tensor_copy(c2hi_f[:], c2rows[0:1, :])
    c2lo_f = const.tile([1, C], F32)
    nc.vector.tensor_tensor(c2lo_f[:], c2row_f[:], c2hi_f[:], op=ALU.subtract)
    # engines can't write at base partition 1; a casting SBUF->SBUF DMA can
    nc.gpsimd.dma_start(c2rows[1:2, :], c2lo_f[:])

    ones2 = const.tile([2, C], BF16)
    nc.vector.memset(ones2[:], 1.0)
    ones_col = const.tile([P, 1], F32)
    nc.vector.memset(ones_col[:], 1.0)

    # c2rows replicated PSUM_GROUP times for the single N=512 c2 matmul
    c2rows4 = const.tile([2, PSUM_GROUP, C], BF16)
    for i in range(PSUM_GROUP):
        nc.vector.tensor_copy(c2rows4[:, i, :], c2rows[:])

    # all-ones [d, c] rhs for the PE-side x2 rank-1 accumulation
    ones_dc = const.tile([P, C], BF16)
    nc.vector.memset(ones_dc[:], 1.0)

    # semi_target, laid out to match the x row mapping below:
    # batch row i = g*(DMA_GROUP*P) + p*DMA_GROUP + t  lives at
    # st_sb[p, g*DMA_GROUP + t]
    st_sb = const.tile([P, NT], F32)
    nc.gpsimd.dma_start(
        st_sb[:].rearrange("p (g t) -> p g t", t=DMA_GROUP),
        st_d.rearrange("(g p t) -> p g t", p=P, t=DMA_GROUP),
    )

    # per-b-tile accumulators: column j <-> b-tile j, partition p <-> row in tile
    mw = const.tile([P, NT], F32)
    x2w = const.tile([P, NT], F32)
    lsum2 = const.tile([P, 2], F32)
    if X2_PE_OF_16 > 0:
        # PE-x2 groups fold x2 into G before the min; their x2w columns
        # must read as zero in the endgame's dist = x2w + mw
        nc.vector.memset(x2w[:], 0.0)

    # ---- endgame (runs per half; first half overlaps the main loop) ----
    def endgame_half(h):
        cols = slice(h * (NT // 2), (h + 1) * (NT // 2))
        W = NT // 2
        dist = endp.tile([P, W], F32, tag=f"dist{h}")
        nc.vector.tensor_tensor(dist[:], x2w[:, cols], mw[:, cols], op=ALU.add)
        nc.vector.tensor_scalar_max(dist[:], dist[:], 0.0)
        dp = endp.tile([P, W], F32, tag=f"dp{h}")
        nc.vector.tensor_scalar_add(dp[:], dist[:], EPS)
        r = endp.tile([P, W], F32, tag=f"r{h}")
        nc.vector.reciprocal(r[:], dp[:])
        # loss = dist + min(st,0)*(dist - r) + max(st,0)*EPS
        t1 = endp.tile([P, W], F32, tag=f"t1{h}")
        nc.vector.tensor_tensor(t1[:], dist[:], r[:], op=ALU.subtract)
        mneg = endp.tile([P, W], F32, tag=f"mneg{h}")
        nc.vector.tensor_scalar_min(mneg[:], st_sb[:, cols], 0.0)
        t2 = endp.tile([P, W], F32, tag=f"t2{h}")
        nc.vector.tensor_tensor(t2[:], mneg[:], t1[:], op=ALU.mult)
        t3 = endp.tile([P, W], F32, tag=f"t3{h}")
        nc.vector.tensor_tensor(t3[:], dist[:], t2[:], op=ALU.add)
        epsq = endp.tile([P, W], F32, tag=f"eq{h}")
        nc.vector.tensor_scalar(epsq[:], st_sb[:, cols], 0.0, EPS, op0=ALU.max,
                                op1=ALU.mult)
        losses = endp.tile([P, W], F32, tag=f"lo{h}")
        nc.vector.tensor_tensor(losses[:], t3[:], epsq[:], op=ALU.add)
        nc.vector.tensor_reduce(lsum2[:, h:h + 1], losses[:],
                                axis=mybir.AxisListType.X, op=ALU.add)

    # ---- main loop (repeat/hw_loop >1 only for steady-state benchmarking) ----
    group_idx = 0
    with tc.For_i(0, hw_loop, 1) if hw_loop > 1 else nullcontext():
     for _rep in range(repeat):
      for gd in range(NT // DMA_GROUP):
        src = x_d[gd * DMA_GROUP * P:(gd + 1) * DMA_GROUP * P, :]
        # row (p, t) of this group = batch gd*1024 + p*8 + t: each partition
        # reads one contiguous 8 KiB run per DMA
        src = src.rearrange("(p t) d -> p t d", t=DMA_GROUP)
        xf8 = xpool.tile([P, DMA_GROUP, D], F32, tag="xf")
        nc.sync.dma_start(xf8[:], src)
        x8 = xbpool.tile([P, DMA_GROUP, D], BF16, tag="xb")
        gmod = gd % 16
        n_dve = 16 - CAST_GPS_OF_16 - CAST_ACT_OF_16
        if gmod >= 16 - CAST_GPS_OF_16:
            nc.gpsimd.tensor_copy(x8[:], xf8[:])
        elif n_dve and gmod % max(1, 16 // max(n_dve, 1)) == 0 and (
                gmod // max(1, 16 // max(n_dve, 1))) < n_dve:
            nc.vector.tensor_copy(x8[:], xf8[:])
        else:
            nc.scalar.copy(x8[:], xf8[:])

        cols = slice(gd * DMA_GROUP, (gd + 1) * DMA_GROUP)
        x2_on_pe = (gd % 3 != 2) if X2_PE_OF_16 == 11 else (gd % 16) < X2_PE_OF_16
        if x2_on_pe:
            pass
        elif X2_MODE == "sqred":
            sq = sqpool.tile([P, DMA_GROUP, D], BF16, tag="sq")
            if SQ_ENG == "act":
                nc.scalar.activation(sq[:], xf8[:], AF.Square)
            else:
                nc.vector.scalar_tensor_tensor(
                    out=sq[:], in0=x8[:], scalar=1.0, in1=x8[:],
                    op0=ALU.mult, op1=ALU.mult,
                )
            nc.vector.tensor_reduce(
                x2w[:, cols], sq[:], axis=mybir.AxisListType.X, op=ALU.add,
            )
        else:
            for t in range(DMA_GROUP):
                col = gd * DMA_GROUP + t
                if (col % 10) < X2_DVE_OF_10:
                    sq = sqpool.tile([P, D], BF16, tag="sqd")
                    nc.vector.scalar_tensor_tensor(
                        out=sq[:], in0=x8[:, t, :], scalar=1.0, in1=x8[:, t, :],
                        op0=ALU.mult, op1=ALU.mult,
                        accum_out=x2w[:, col:col + 1],
                    )
                else:
                    sq = sqpool.tile([P, D], F32, tag="sqa")
                    nc.scalar.activation(
                        sq[:], x8[:, t, :], AF.Square,
                        accum_out=x2w[:, col:col + 1],
                    )

        if TPOSE_DMA:
            xtg = xtsb.tile([P, DMA_GROUP * 2, P], BF16)
            nc.scalar.dma_start_transpose(
                xtg[:], x8[:].rearrange("p t d -> p (t d)")
            )

        g_ps = gps.tile([P, DMA_GROUP, C], F32)
        for gp in range(DMA_GROUP // PSUM_GROUP):
            tiles = [gp * PSUM_GROUP + t for t in range(PSUM_GROUP)]

            if TPOSE_DMA:
                xt_view = xtg[:].rearrange("p (t k) b -> p t k b", k=2)
                xt_sb = xt_view[:, gp * PSUM_GROUP:(gp + 1) * PSUM_GROUP, :, :]
            else:
                xt_ps = xtps.tile([P, PSUM_GROUP, 2, P], BF16)
                for i, t in enumerate(tiles):
                    for k in range(2):
                        nc.tensor.transpose(
                            xt_ps[:, i, k, :], x8[:, t, k * P:(k + 1) * P],
                            ident_bf[:],
                        )
                xt_t = xtsb.tile([P, PSUM_GROUP, 2, P], BF16)
                # bf16 stays bf16 in PSUM; move it as fp32 pairs (half the
                # elements; exact on normals)
                cp_src = xt_ps[:].bitcast(F32)
                cp_dst = xt_t[:].bitcast(F32)
                if (group_idx % 8) < COPY_DVE_OF_8:
                    nc.vector.tensor_copy(cp_dst, cp_src)
                else:
                    nc.scalar.copy(cp_dst, cp_src)
                xt_sb = xt_t[:]

            if x2_on_pe:
                sqt = sqpool.tile([P, PSUM_GROUP, 2, P], BF16, tag="sqt")
                nc.vector.tensor_tensor(sqt[:], xt_t[:], xt_t[:], op=ALU.mult)

            g_half = g_ps[:, gp * PSUM_GROUP:(gp + 1) * PSUM_GROUP, :]
            nc.tensor.matmul(
                g_half.rearrange("p t c -> p (t c)"),
                lhsT=ones2[:], rhs=c2rows4[:].rearrange("p t c -> p (t c)"),
                start=True, stop=False,
            )
            for i in range(PSUM_GROUP):
                last_tile = i == PSUM_GROUP - 1
                nc.tensor.matmul(
                    g_half[:, i, :], lhsT=xt_sb[:, i, 0, :], rhs=cT[:, 0, :],
                    start=False, stop=False,
                )
                nc.tensor.matmul(
                    g_half[:, i, :], lhsT=xt_sb[:, i, 1, :], rhs=cT[:, 1, :],
                    start=False, stop=(last_tile and not x2_on_pe),
                )
                if x2_on_pe:
                    nc.tensor.matmul(
                        g_half[:, i, :], lhsT=sqt[:, i, 0, :], rhs=ones_dc[:],
                        start=False, stop=False,
                    )
                    nc.tensor.matmul(
                        g_half[:, i, :], lhsT=sqt[:, i, 1, :], rhs=ones_dc[:],
                        start=False, stop=last_tile,
                    )
            group_idx += 1

        nc.vector.tensor_reduce(
            mw[:, cols], g_ps[:], axis=mybir.AxisListType.X, op=ALU.min,
        )
        if repeat == 1 and hw_loop == 1 and gd == NT // DMA_GROUP // 2 - 1:
            endgame_half(0)

    endgame_half(1)
    lsum = endp.tile([P, 1], F32)
    nc.vector.tensor_tensor(lsum[:], lsum2[:, 0:1], lsum2[:, 1:2], op=ALU.add)
    total_ps = scr_ps.tile([1, 1], F32, tag="scratch")
    nc.tensor.matmul(total_ps[:], lhsT=ones_col[:], rhs=lsum[:])
    total_sb = endp.tile([1, 1], F32)
    nc.vector.tensor_copy(total_sb[:], total_ps[:])
    nc.sync.dma_start(out_d[:], total_sb[:])


def build_nc(repeat: int = 1, hw_loop: int = 1, internal_x: bool = False):
    key = (repeat, hw_loop, internal_x)
    if key in _cached_nc:
        return _cached_nc[key]
    nc = bacc.Bacc(
        "TRN2",
        target_bir_lowering=False,
        debug=False,
        enable_asserts=False,
        num_devices=N_CORES,
    )
    if internal_x:
        # timing-only builds: x is internal (uninitialized) DRAM so bench
        # calls don't upload 128 MiB; compute timing is data-independent
        x_d = nc.dram_tensor("x", [B_SH, D], F32).ap()
    else:
        x_d = nc.dram_tensor("x", [B_SH, D], F32, kind="ExternalInput").ap()
    c_d = nc.dram_tensor("c", [C, D], F32, kind="ExternalInput").ap()
    st_d = nc.dram_tensor("st", [B_SH], F32, kind="ExternalInput").ap()
    out_d = nc.dram_tensor("out", [1, 1], F32, kind="ExternalOutput").ap()

    with tile.TileContext(nc) as tc:
        with ExitStack() as ctx:
            _emit(ctx, tc, x_d, c_d, st_d, out_d, repeat=repeat, hw_loop=hw_loop)
    nc.compile()
    _cached_nc[key] = nc
    return nc


def make_in_maps(x, c, stf):
    return [
        {
            "x": np.ascontiguousarray(x[i * B_SH:(i + 1) * B_SH]),
            "c": c,
            "st": np.ascontiguousarray(stf[i * B_SH:(i + 1) * B_SH]),
        }
        for i in range(N_CORES)
    ]


def kernel(**inputs) -> np.ndarray:
    x = np.ascontiguousarray(np.asarray(inputs["input"], dtype=np.float32))
    c = np.ascontiguousarray(np.asarray(inputs["c"], dtype=np.float32))
    stf = np.asarray(inputs["semi_target"]).astype(np.float32)

    nc = build_nc()
    res = run_bass_kernel_spmd(nc, make_in_maps(x, c, stf), list(range(N_CORES)))
    total = sum(float(r["out"][0, 0]) for r in res.results)
    return np.asarray(np.float32(total / B))
